# revision 3
# baseline (speedup 1.0000x reference)
"""Multi-head attention (RoPE, softmax, out-proj) on 8 Trainium2 NeuronCores.

Sharding: batch (2) x head-groups (4) -> 8 cores. Each core computes, for its
batch b and its 4 heads: q/k/v projections (column-parallel), RoPE, full
attention, and a partial output projection against its slice of wo
(row-parallel). The 4 partial outputs per batch are summed on the host.

All big matmuls run in float32r (full PE rate at N>=256, ~1e-4 rel err).

Layout trick: weights are pre-transposed on the host so every matmul operand
is a natural [contraction-dim-major] DMA. Within each head, q/k feature rows
are permuted to (even pairs, odd pairs) so RoPE's interleaved pair structure
becomes a partition-block structure (rows 0:64 / 64:128); scores are
invariant to the (shared) permutation and v/wo stay unpermuted.

Softmax is computed unnormalized (exp without max subtraction is safe here:
scores ~ N(0,1)); the denominator comes from an ones-matmul (partition-dim
reduction on the PE, broadcast across all 128 rows for free), and the
normalization multiply happens on the transposed attention output where sq is
the free dim.
"""
import math
import sys

import numpy as np

for _p in ('/opt/trn_rl_repo', '/root/.axon_site/_ro/trn_rl_repo'):
    if _p not in sys.path:
        sys.path.insert(0, _p)

import orjson

import concourse.bass as bass
import concourse.mybir as mybir
from concourse.tile import TileContext
from concourse.bass_utils import run_bass_kernel_spmd

F32 = mybir.dt.float32
R32 = mybir.dt.float32r

B = 2
S = 2048
D = 2048
HD = 128
N_CORES = 8
GROUPS = 4          # head groups (tensor-parallel degree per batch)
HPC = (D // HD) // GROUPS  # heads per core (4)
LF = HPC * HD       # local features per core (512)


# ---------------------------------------------------------------------------
# Wait-splitting post-pass: this toolchain's walrus supports at most ONE sync
# wait command per instruction (none on fp32/fp32r Matmult, which lowers to an
# LDW+MM pair). Tile emits multi-wait instructions; hoist the excess onto NoOps
# on the same engine immediately before the instruction.
# ---------------------------------------------------------------------------

def _keep_count(ins):
    if ins.get('opcode') == 'Matmult':
        dt = None
        for arg in ins.get('ins', []):
            dt = arg.get('dtype') or dt
        if dt in ('float32', 'float32r'):
            return 0
        return 1
    return 1


def _split_waits_json(data: bytes) -> bytes:
    d = orjson.loads(data)
    ctr = 0
    for fn in d.get('functions', []):
        for bb in fn.get('blocks', []):
            out = []
            for ins in bb.get('instructions', []):
                si = ins.get('sync_info')
                waits = (si or {}).get('on_wait') or []
                keep = _keep_count(ins)
                if len(waits) > keep:
                    hoist = waits[:len(waits) - keep]
                    keep_w = waits[len(waits) - keep:]
                    for w in hoist:
                        ctr += 1
                        nop = {
                            'name': f"{ins['name']}-ws{ctr}",
                            'opcode': 'NoOp',
                            'engine': ins.get('engine'),
                            'ins': [],
                            'outs': [],
                            'sync_info': {'on_wait': [w], 'on_update': []},
                        }
                        if 'debug' in ins:
                            nop['debug'] = ins['debug']
                        out.append(nop)
                    si['on_wait'] = keep_w
                out.append(ins)
            bb['instructions'] = out
    return orjson.dumps(d)


def _install_waitsplit():
    if getattr(bass.Bass, '_waitsplit_installed', False):
        return
    orig = bass.Bass.to_json_bytes

    def patched(self, *a, **k):
        return _split_waits_json(orig(self, *a, **k))

    bass.Bass.to_json_bytes = patched
    bass.Bass._waitsplit_installed = True


_install_waitsplit()


# ---------------------------------------------------------------------------
# Device program (SPMD, identical on all cores; per-core data differs)
# ---------------------------------------------------------------------------

def build_nc(s=S, d=D, hpc=HPC):
    lf = hpc * HD
    kd_n = d // 128          # contraction chunks for projections
    nw = 512 if s >= 512 else s  # free-dim width per matmul
    nsq = s // nw            # wide column chunks
    ns = s // 128            # 128-row chunks
    nj = d // 512 if d >= 512 else 1
    jw = 512 if d >= 512 else d
    scale = 1.0 / math.sqrt(HD)

    nc = bass.Bass()
    xT = nc.dram_tensor("xT", [d, s], F32, kind="ExternalInput")
    wqT = nc.dram_tensor("wqT", [d, lf], F32, kind="ExternalInput")
    wkT = nc.dram_tensor("wkT", [d, lf], F32, kind="ExternalInput")
    wvT = nc.dram_tensor("wvT", [d, lf], F32, kind="ExternalInput")
    woT = nc.dram_tensor("woT", [lf, d], F32, kind="ExternalInput")
    csd = nc.dram_tensor("csd", [128, s], F32, kind="ExternalInput")
    snd = nc.dram_tensor("snd", [128, s], F32, kind="ExternalInput")
    y = nc.dram_tensor("y", [s, d], F32, kind="ExternalOutput")

    qT_d = nc.dram_tensor("qT_d", [lf, s], F32)
    kT_d = nc.dram_tensor("kT_d", [lf, s], F32)
    v_d = nc.dram_tensor("v_d", [s, lf], F32)
    aT_d = nc.dram_tensor("aT_d", [lf, s], F32)

    with TileContext(nc) as tc:
        # ---------------- Stage A1: q/k projections + RoPE ----------------
        with tc.tile_pool(name="wqk", bufs=1) as wpool, \
             tc.tile_pool(name="xa", bufs=2) as xpool, \
             tc.tile_pool(name="csp", bufs=1) as cspool, \
             tc.tile_pool(name="rp", bufs=3) as rpool, \
             tc.tile_pool(name="psA", bufs=3, space="PSUM") as pspool:
            wq_sb = wpool.tile([128, kd_n * lf], R32, name="wq_sb")
            wk_sb = wpool.tile([128, kd_n * lf], R32, name="wk_sb")
            for kd in range(kd_n):
                nc.sync.dma_start(out=wq_sb[:, kd * lf:(kd + 1) * lf],
                                  in_=wqT[kd * 128:(kd + 1) * 128, :].bitcast(R32))
                nc.sync.dma_start(out=wk_sb[:, kd * lf:(kd + 1) * lf],
                                  in_=wkT[kd * 128:(kd + 1) * 128, :].bitcast(R32))
            cs_sb = cspool.tile([128, s], F32, name="cs_sb")
            sn_sb = cspool.tile([128, s], F32, name="sn_sb")
            nc.sync.dma_start(out=cs_sb, in_=csd[:, :])
            nc.sync.dma_start(out=sn_sb, in_=snd[:, :])
            for sq in range(nsq):
                x_sb = xpool.tile([128, kd_n * nw], R32, name="x_sb")
                for kd in range(kd_n):
                    nc.sync.dma_start(
                        out=x_sb[:, kd * nw:(kd + 1) * nw],
                        in_=xT[kd * 128:(kd + 1) * 128, sq * nw:(sq + 1) * nw].bitcast(R32))
                for wsb, dst in ((wq_sb, qT_d), (wk_sb, kT_d)):
                    for h in range(hpc):
                        ps = pspool.tile([128, nw], F32, name="ps_qk")
                        for kd in range(kd_n):
                            nc.tensor.matmul(
                                ps,
                                wsb[:, kd * lf + h * 128: kd * lf + (h + 1) * 128],
                                x_sb[:, kd * nw:(kd + 1) * nw],
                                start=(kd == 0), stop=(kd == kd_n - 1))
                        tcc = rpool.tile([128, nw], F32, name="t_c")
                        tss = rpool.tile([128, nw], F32, name="t_s")
                        nc.vector.tensor_mul(tcc, ps, cs_sb[:, sq * nw:(sq + 1) * nw])
                        # sn_sb rows are [+sin; -sin], so after the half-swap the
                        # signed cross terms land with the right signs
                        nc.vector.tensor_mul(tss, ps, sn_sb[:, sq * nw:(sq + 1) * nw])
                        tsw = rpool.tile([128, nw], F32, name="t_sw")
                        nc.sync.dma_start(out=tsw[0:64, :], in_=tss[64:128, :])
                        nc.sync.dma_start(out=tsw[64:128, :], in_=tss[0:64, :])
                        ro = rpool.tile([128, nw], F32, name="ro")
                        nc.vector.tensor_add(ro, tcc, tsw)
                        nc.sync.dma_start(
                            out=dst[h * 128:(h + 1) * 128, sq * nw:(sq + 1) * nw], in_=ro)

        # ---------------- Stage A2: v projection ----------------
        with tc.tile_pool(name="wvp", bufs=1) as wvpool, \
             tc.tile_pool(name="xa2", bufs=2) as xpool2, \
             tc.tile_pool(name="vop", bufs=3) as vopool, \
             tc.tile_pool(name="psA2", bufs=3, space="PSUM") as pspool2:
            wv_sb = wvpool.tile([128, kd_n * lf], R32, name="wv_sb")
            for kd in range(kd_n):
                nc.sync.dma_start(out=wv_sb[:, kd * lf:(kd + 1) * lf],
                                  in_=wvT[kd * 128:(kd + 1) * 128, :].bitcast(R32))
            for sq in range(nsq):
                x_sb2 = xpool2.tile([128, kd_n * nw], R32, name="x_sb2")
                for kd in range(kd_n):
                    nc.sync.dma_start(
                        out=x_sb2[:, kd * nw:(kd + 1) * nw],
                        in_=xT[kd * 128:(kd + 1) * 128, sq * nw:(sq + 1) * nw].bitcast(R32))
                for ss in range(nw // 128):
                    ps = pspool2.tile([128, lf], F32, name="ps_v")
                    for kd in range(kd_n):
                        nc.tensor.matmul(
                            ps,
                            x_sb2[:, kd * nw + ss * 128: kd * nw + (ss + 1) * 128],
                            wv_sb[:, kd * lf:(kd + 1) * lf],
                            start=(kd == 0), stop=(kd == kd_n - 1))
                    vo = vopool.tile([128, lf], F32, name="vo")
                    nc.vector.tensor_copy(vo, ps)
                    nc.sync.dma_start(
                        out=v_d[sq * nw + ss * 128: sq * nw + (ss + 1) * 128, :], in_=vo)

        # ---------------- Stage B: attention per head ----------------
        with tc.tile_pool(name="kv", bufs=2) as kvpool, \
             tc.tile_pool(name="qb", bufs=2) as qpool, \
             tc.tile_pool(name="exp", bufs=2) as expool, \
             tc.tile_pool(name="onep", bufs=1) as onepool, \
             tc.tile_pool(name="nrm", bufs=3) as npool, \
             tc.tile_pool(name="psS", bufs=2, space="PSUM") as pssc, \
             tc.tile_pool(name="psM", bufs=2, space="PSUM") as pssm, \
             tc.tile_pool(name="psV", bufs=2, space="PSUM") as psov:
            ones_f = onepool.tile([128, 128], F32, name="ones_f")
            nc.vector.memset(ones_f, 1.0)
            ones = onepool.tile([128, 128], R32, name="ones")
            nc.vector.tensor_copy(ones, ones_f)
            for h in range(hpc):
                kT_sb = kvpool.tile([128, s], R32, name="kT_sb")
                nc.sync.dma_start(out=kT_sb, in_=kT_d[h * 128:(h + 1) * 128, :].bitcast(R32))
                v_sb = kvpool.tile([128, ns * 128], R32, name="v_sb")
                for sk in range(ns):
                    nc.sync.dma_start(
                        out=v_sb[:, sk * 128:(sk + 1) * 128],
                        in_=v_d[sk * 128:(sk + 1) * 128, h * 128:(h + 1) * 128].bitcast(R32))
                for sq in range(nsq):
                    qT_sb = qpool.tile([128, nw], R32, name="qT_sb")
                    nc.sync.dma_start(
                        out=qT_sb,
                        in_=qT_d[h * 128:(h + 1) * 128, sq * nw:(sq + 1) * nw].bitcast(R32))
                    ex_sb = expool.tile([128, ns * nw], R32, name="ex_sb")
                    for sk in range(ns):
                        sps = pssc.tile([128, nw], F32, name="sps")
                        nc.tensor.matmul(sps, kT_sb[:, sk * 128:(sk + 1) * 128], qT_sb,
                                         start=True, stop=True)
                        nc.scalar.activation(ex_sb[:, sk * nw:(sk + 1) * nw], sps,
                                             mybir.ActivationFunctionType.Exp, scale=scale)
                    sm = pssm.tile([128, nw], F32, name="sm")
                    for sk in range(ns):
                        nc.tensor.matmul(sm, ones, ex_sb[:, sk * nw:(sk + 1) * nw],
                                         start=(sk == 0), stop=(sk == ns - 1))
                    ov = psov.tile([128, nw], F32, name="ov")
                    for sk in range(ns):
                        nc.tensor.matmul(ov, v_sb[:, sk * 128:(sk + 1) * 128],
                                         ex_sb[:, sk * nw:(sk + 1) * nw],
                                         start=(sk == 0), stop=(sk == ns - 1))
                    rec = npool.tile([128, nw], F32, name="rec")
                    nc.vector.reciprocal(rec, sm)
                    ao = npool.tile([128, nw], F32, name="ao")
                    nc.vector.tensor_mul(ao, ov, rec)
                    nc.sync.dma_start(
                        out=aT_d[h * 128:(h + 1) * 128, sq * nw:(sq + 1) * nw], in_=ao)

        # ---------------- Stage C: output projection (partial) ----------------
        with tc.tile_pool(name="wop", bufs=1) as wopool, \
             tc.tile_pool(name="acp", bufs=2) as acpool, \
             tc.tile_pool(name="yop", bufs=3) as yopool, \
             tc.tile_pool(name="psC", bufs=4, space="PSUM") as psc:
            wo_sb = wopool.tile([128, hpc * d], R32, name="wo_sb")
            for i in range(hpc):
                nc.sync.dma_start(out=wo_sb[:, i * d:(i + 1) * d],
                                  in_=woT[i * 128:(i + 1) * 128, :].bitcast(R32))
            for sq in range(ns):
                a_sb = acpool.tile([128, hpc * 128], R32, name="a_sb")
                for i in range(hpc):
                    nc.sync.dma_start(
                        out=a_sb[:, i * 128:(i + 1) * 128],
                        in_=aT_d[i * 128:(i + 1) * 128, sq * 128:(sq + 1) * 128].bitcast(R32))
                for jn in range(nj):
                    yps = psc.tile([128, jw], F32, name="yps")
                    for i in range(hpc):
                        nc.tensor.matmul(yps, a_sb[:, i * 128:(i + 1) * 128],
                                         wo_sb[:, i * d + jn * jw: i * d + (jn + 1) * jw],
                                         start=(i == 0), stop=(i == hpc - 1))
                    yo = yopool.tile([128, jw], F32, name="yo")
                    nc.vector.tensor_copy(yo, yps)
                    nc.sync.dma_start(
                        out=y[sq * 128:(sq + 1) * 128, jn * jw:(jn + 1) * jw], in_=yo)
    return nc


# ---------------------------------------------------------------------------
# Host-side sharding + gather
# ---------------------------------------------------------------------------

_PERM_HEAD = np.concatenate([np.arange(0, HD, 2), np.arange(1, HD, 2)])


def _prep_in_maps(x, wq, wk, wv, wo, pos_cos, pos_sin, s=S, d=D, hpc=HPC):
    lf = hpc * HD
    h_total = d // HD
    groups = h_total // hpc
    # permute q/k feature rows within each head: even pairs first, then odd
    wq_p = wq.reshape(h_total, HD, d)[:, _PERM_HEAD, :].reshape(d, d)
    wk_p = wk.reshape(h_total, HD, d)[:, _PERM_HEAD, :].reshape(d, d)
    wqT_full = np.ascontiguousarray(wq_p.T)
    wkT_full = np.ascontiguousarray(wk_p.T)
    wvT_full = np.ascontiguousarray(wv.T)
    woT_full = np.ascontiguousarray(wo.T)
    cs_half = np.ascontiguousarray(pos_cos[0].T)  # [64, S]
    sn_half = np.ascontiguousarray(pos_sin[0].T)
    csd = np.concatenate([cs_half, cs_half], axis=0)
    snd = np.concatenate([sn_half, -sn_half], axis=0)
    in_maps = []
    n_batches = x.shape[0]
    for c in range(n_batches * groups):
        b, g = divmod(c, groups)
        in_maps.append({
            "xT": np.ascontiguousarray(x[b].T),
            "wqT": np.ascontiguousarray(wqT_full[:, g * lf:(g + 1) * lf]),
            "wkT": np.ascontiguousarray(wkT_full[:, g * lf:(g + 1) * lf]),
            "wvT": np.ascontiguousarray(wvT_full[:, g * lf:(g + 1) * lf]),
            "woT": np.ascontiguousarray(woT_full[g * lf:(g + 1) * lf, :]),
            "csd": csd,
            "snd": snd,
        })
    return in_maps


_NC_CACHE = {}


def _get_nc(s=S, d=D, hpc=HPC):
    key = (s, d, hpc)
    if key not in _NC_CACHE:
        _NC_CACHE[key] = build_nc(s, d, hpc)
    return _NC_CACHE[key]


def kernel(x, wq, wk, wv, wo, pos_cos, pos_sin):
    x = np.asarray(x, dtype=np.float32)
    in_maps = _prep_in_maps(np.asarray(x), np.asarray(wq), np.asarray(wk),
                            np.asarray(wv), np.asarray(wo),
                            np.asarray(pos_cos), np.asarray(pos_sin))
    nc = _get_nc()
    res = run_bass_kernel_spmd(nc, in_maps, core_ids=list(range(N_CORES)))
    out = np.empty((B, S, D), dtype=np.float32)
    for b in range(B):
        acc = res.results[b * GROUPS]["y"].astype(np.float32)
        for g in range(1, GROUPS):
            acc = acc + res.results[b * GROUPS + g]["y"]
        out[b] = acc
    return out


# revision 4
# speedup vs baseline: 1.5718x; 1.5718x over previous
"""Multi-head attention (RoPE, softmax, out-proj) on 8 Trainium2 NeuronCores.

Sharding: batch (2) x head-groups (4) -> 8 cores. Each core computes, for its
batch b and its 4 heads: q/k/v projections (column-parallel), RoPE, full
attention, and a partial output projection against its slice of wo
(row-parallel). The 4 partial outputs per batch are summed on the host.

Matmuls run in bf16 (full PE rate, FWL weight loads) with fp32 PSUM
accumulation; the softmax denominator path runs in fp32/fp32r so the
normalization carries no bf16 systematic error.

Layout trick: weights are pre-transposed on the host so every matmul operand
is a natural [contraction-dim-major] DMA. Within each head, q/k feature rows
are permuted to (even pairs, odd pairs) so RoPE's interleaved pair structure
becomes a partition-block structure (rows 0:64 / 64:128); scores are
invariant to the (shared) permutation and v/wo stay unpermuted. The halves
swap needed by RoPE's cross terms is done with two SBUF->SBUF DMAs and the
signs are folded into the (host-prepared) sin rows [+sin; -sin].

Softmax is computed unnormalized (exp without max subtraction is safe:
scores ~ N(0,1)). The denominator: exp tiles are accumulated across
key-chunks on the DVE (fp32), then one ones-matmul per query chunk reduces
over partitions and broadcasts the row of sums to all 128 partitions; the
reciprocal multiply happens on the transposed attention output where the
query index is the free dim.
"""
import math
import sys

import numpy as np

for _p in ('/opt/trn_rl_repo', '/root/.axon_site/_ro/trn_rl_repo'):
    if _p not in sys.path:
        sys.path.insert(0, _p)

import ml_dtypes
import orjson

import concourse.bass as bass
import concourse.mybir as mybir
from concourse.tile import TileContext
from concourse.bass_utils import run_bass_kernel_spmd

F32 = mybir.dt.float32
R32 = mybir.dt.float32r
BF16 = mybir.dt.bfloat16
NP_BF16 = ml_dtypes.bfloat16

B = 2
S = 2048
D = 2048
HD = 128
N_CORES = 8
GROUPS = 4          # head groups (tensor-parallel degree per batch)
HPC = (D // HD) // GROUPS  # heads per core (4)
LF = HPC * HD       # local features per core (512)


# ---------------------------------------------------------------------------
# Wait-splitting post-pass: this toolchain's walrus supports at most ONE sync
# wait command per instruction (none at all on fp32/fp32r Matmult, which
# lowers to an LDW+MM pair). Tile emits multi-wait instructions; hoist the
# excess onto NoOps on the same engine immediately before the instruction.
# ---------------------------------------------------------------------------

def _keep_count(ins):
    if ins.get('opcode') == 'Matmult':
        dt = None
        for arg in ins.get('ins', []):
            dt = arg.get('dtype') or dt
        if dt in ('float32', 'float32r'):
            return 0
        return 1
    return 1


def _split_waits_json(data: bytes) -> bytes:
    d = orjson.loads(data)
    ctr = 0
    for fn in d.get('functions', []):
        for bb in fn.get('blocks', []):
            out = []
            for ins in bb.get('instructions', []):
                si = ins.get('sync_info')
                waits = (si or {}).get('on_wait') or []
                keep = _keep_count(ins)
                if len(waits) > keep:
                    hoist = waits[:len(waits) - keep]
                    keep_w = waits[len(waits) - keep:]
                    for w in hoist:
                        ctr += 1
                        nop = {
                            'name': f"{ins['name']}-ws{ctr}",
                            'opcode': 'NoOp',
                            'engine': ins.get('engine'),
                            'ins': [],
                            'outs': [],
                            'sync_info': {'on_wait': [w], 'on_update': []},
                        }
                        if 'debug' in ins:
                            nop['debug'] = ins['debug']
                        out.append(nop)
                    si['on_wait'] = keep_w
                out.append(ins)
            bb['instructions'] = out
    return orjson.dumps(d)


def _install_waitsplit():
    if getattr(bass.Bass, '_waitsplit_installed', False):
        return
    orig = bass.Bass.to_json_bytes

    def patched(self, *a, **k):
        return _split_waits_json(orig(self, *a, **k))

    bass.Bass.to_json_bytes = patched
    bass.Bass._waitsplit_installed = True


_install_waitsplit()


# ---------------------------------------------------------------------------
# Device program (SPMD, identical on all cores; per-core data differs)
# ---------------------------------------------------------------------------

def build_nc(s=S, d=D, hpc=HPC):
    lf = hpc * HD
    kd_n = d // 128          # contraction chunks for projections
    nw = 512 if s >= 512 else s  # free-dim width per matmul
    nsq = s // nw            # wide column chunks
    ns = s // 128            # 128-row chunks
    nj = d // 512 if d >= 512 else 1
    jw = 512 if d >= 512 else d
    scale = 1.0 / math.sqrt(HD)

    nc = bass.Bass()
    xT = nc.dram_tensor("xT", [d, s], BF16, kind="ExternalInput")
    wqT = nc.dram_tensor("wqT", [d, lf], BF16, kind="ExternalInput")
    wkT = nc.dram_tensor("wkT", [d, lf], BF16, kind="ExternalInput")
    wvT = nc.dram_tensor("wvT", [d, lf], BF16, kind="ExternalInput")
    woT = nc.dram_tensor("woT", [lf, d], BF16, kind="ExternalInput")
    csd = nc.dram_tensor("csd", [128, s], F32, kind="ExternalInput")
    snd = nc.dram_tensor("snd", [128, s], F32, kind="ExternalInput")
    y = nc.dram_tensor("y", [s, d], F32, kind="ExternalOutput")

    qT_d = nc.dram_tensor("qT_d", [lf, s], BF16)
    kT_d = nc.dram_tensor("kT_d", [lf, s], BF16)
    v_d = nc.dram_tensor("v_d", [s, lf], BF16)
    aT_d = nc.dram_tensor("aT_d", [lf, s], BF16)

    with TileContext(nc) as tc:
        # ---------------- Stage A1: q/k projections + RoPE ----------------
        with tc.tile_pool(name="wqk", bufs=1) as wpool, \
             tc.tile_pool(name="xa", bufs=2) as xpool, \
             tc.tile_pool(name="csp", bufs=1) as cspool, \
             tc.tile_pool(name="rp", bufs=3) as rpool, \
             tc.tile_pool(name="psA", bufs=3, space="PSUM") as pspool:
            wq_sb = wpool.tile([128, kd_n * lf], BF16, name="wq_sb")
            wk_sb = wpool.tile([128, kd_n * lf], BF16, name="wk_sb")
            for kd in range(kd_n):
                nc.sync.dma_start(out=wq_sb[:, kd * lf:(kd + 1) * lf],
                                  in_=wqT[kd * 128:(kd + 1) * 128, :])
                nc.sync.dma_start(out=wk_sb[:, kd * lf:(kd + 1) * lf],
                                  in_=wkT[kd * 128:(kd + 1) * 128, :])
            cs_sb = cspool.tile([128, s], F32, name="cs_sb")
            sn_sb = cspool.tile([128, s], F32, name="sn_sb")
            nc.sync.dma_start(out=cs_sb, in_=csd[:, :])
            nc.sync.dma_start(out=sn_sb, in_=snd[:, :])
            for sq in range(nsq):
                x_sb = xpool.tile([128, kd_n * nw], BF16, name="x_sb")
                for kd in range(kd_n):
                    nc.sync.dma_start(
                        out=x_sb[:, kd * nw:(kd + 1) * nw],
                        in_=xT[kd * 128:(kd + 1) * 128, sq * nw:(sq + 1) * nw])
                for wsb, dst in ((wq_sb, qT_d), (wk_sb, kT_d)):
                    for h in range(hpc):
                        ps = pspool.tile([128, nw], F32, name="ps_qk")
                        for kd in range(kd_n):
                            nc.tensor.matmul(
                                ps,
                                wsb[:, kd * lf + h * 128: kd * lf + (h + 1) * 128],
                                x_sb[:, kd * nw:(kd + 1) * nw],
                                start=(kd == 0), stop=(kd == kd_n - 1))
                        tcc = rpool.tile([128, nw], F32, name="t_c")
                        tss = rpool.tile([128, nw], F32, name="t_s")
                        nc.vector.tensor_mul(tcc, ps, cs_sb[:, sq * nw:(sq + 1) * nw])
                        # sn_sb rows are [+sin; -sin], so after the half-swap
                        # the signed cross terms land with the right signs
                        nc.vector.tensor_mul(tss, ps, sn_sb[:, sq * nw:(sq + 1) * nw])
                        tsw = rpool.tile([128, nw], F32, name="t_sw")
                        nc.sync.dma_start(out=tsw[0:64, :], in_=tss[64:128, :])
                        nc.sync.dma_start(out=tsw[64:128, :], in_=tss[0:64, :])
                        ro = rpool.tile([128, nw], BF16, name="ro")
                        nc.vector.tensor_add(ro, tcc, tsw)
                        nc.sync.dma_start(
                            out=dst[h * 128:(h + 1) * 128, sq * nw:(sq + 1) * nw], in_=ro)

        # ---------------- Stage A2: v projection ----------------
        with tc.tile_pool(name="wvp", bufs=1) as wvpool, \
             tc.tile_pool(name="xa2", bufs=2) as xpool2, \
             tc.tile_pool(name="vop", bufs=3) as vopool, \
             tc.tile_pool(name="psA2", bufs=3, space="PSUM") as pspool2:
            wv_sb = wvpool.tile([128, kd_n * lf], BF16, name="wv_sb")
            for kd in range(kd_n):
                nc.sync.dma_start(out=wv_sb[:, kd * lf:(kd + 1) * lf],
                                  in_=wvT[kd * 128:(kd + 1) * 128, :])
            for sq in range(nsq):
                x_sb2 = xpool2.tile([128, kd_n * nw], BF16, name="x_sb2")
                for kd in range(kd_n):
                    nc.sync.dma_start(
                        out=x_sb2[:, kd * nw:(kd + 1) * nw],
                        in_=xT[kd * 128:(kd + 1) * 128, sq * nw:(sq + 1) * nw])
                for ss in range(nw // 128):
                    ps = pspool2.tile([128, lf], F32, name="ps_v")
                    for kd in range(kd_n):
                        nc.tensor.matmul(
                            ps,
                            x_sb2[:, kd * nw + ss * 128: kd * nw + (ss + 1) * 128],
                            wv_sb[:, kd * lf:(kd + 1) * lf],
                            start=(kd == 0), stop=(kd == kd_n - 1))
                    vo = vopool.tile([128, lf], BF16, name="vo")
                    nc.vector.tensor_copy(vo, ps)
                    nc.sync.dma_start(
                        out=v_d[sq * nw + ss * 128: sq * nw + (ss + 1) * 128, :], in_=vo)

        # ---------------- Stage B: attention per head ----------------
        with tc.tile_pool(name="kv", bufs=2) as kvpool, \
             tc.tile_pool(name="qb", bufs=2) as qpool, \
             tc.tile_pool(name="exp", bufs=2) as expool, \
             tc.tile_pool(name="onep", bufs=1) as onepool, \
             tc.tile_pool(name="nrm", bufs=3) as npool, \
             tc.tile_pool(name="psS", bufs=2, space="PSUM") as pssc, \
             tc.tile_pool(name="psM", bufs=2, space="PSUM") as pssm, \
             tc.tile_pool(name="psV", bufs=2, space="PSUM") as psov:
            ones_f = onepool.tile([128, 128], F32, name="ones_f")
            nc.vector.memset(ones_f, 1.0)
            ones = onepool.tile([128, 128], R32, name="ones")
            nc.vector.tensor_copy(ones, ones_f)
            for h in range(hpc):
                kT_sb = kvpool.tile([128, s], BF16, name="kT_sb")
                nc.sync.dma_start(out=kT_sb, in_=kT_d[h * 128:(h + 1) * 128, :])
                v_sb = kvpool.tile([128, ns * 128], BF16, name="v_sb")
                for sk in range(ns):
                    nc.sync.dma_start(
                        out=v_sb[:, sk * 128:(sk + 1) * 128],
                        in_=v_d[sk * 128:(sk + 1) * 128, h * 128:(h + 1) * 128])
                for sq in range(nsq):
                    qT_sb = qpool.tile([128, nw], BF16, name="qT_sb")
                    nc.sync.dma_start(
                        out=qT_sb,
                        in_=qT_d[h * 128:(h + 1) * 128, sq * nw:(sq + 1) * nw])
                    ex_sb = expool.tile([128, ns * nw], BF16, name="ex_sb")
                    acc = npool.tile([128, nw], F32, name="acc")
                    for sk in range(ns):
                        sps = pssc.tile([128, nw], F32, name="sps")
                        nc.tensor.matmul(sps, kT_sb[:, sk * 128:(sk + 1) * 128], qT_sb,
                                         start=True, stop=True)
                        nc.scalar.activation(ex_sb[:, sk * nw:(sk + 1) * nw], sps,
                                             mybir.ActivationFunctionType.Exp, scale=scale)
                        # fp32 running sum of the exp tiles (across key chunks)
                        if sk == 0:
                            nc.vector.tensor_copy(acc, ex_sb[:, 0:nw])
                        else:
                            nc.vector.tensor_add(acc, acc, ex_sb[:, sk * nw:(sk + 1) * nw])
                    accr = npool.tile([128, nw], R32, name="accr")
                    nc.vector.tensor_copy(accr, acc)
                    # partition reduction + row broadcast of the denominator
                    sm = pssm.tile([128, nw], F32, name="sm")
                    nc.tensor.matmul(sm, ones, accr, start=True, stop=True)
                    ov = psov.tile([128, nw], F32, name="ov")
                    for sk in range(ns):
                        nc.tensor.matmul(ov, v_sb[:, sk * 128:(sk + 1) * 128],
                                         ex_sb[:, sk * nw:(sk + 1) * nw],
                                         start=(sk == 0), stop=(sk == ns - 1))
                    rec = npool.tile([128, nw], F32, name="rec")
                    nc.vector.reciprocal(rec, sm)
                    ao = npool.tile([128, nw], BF16, name="ao")
                    nc.vector.tensor_mul(ao, ov, rec)
                    nc.sync.dma_start(
                        out=aT_d[h * 128:(h + 1) * 128, sq * nw:(sq + 1) * nw], in_=ao)

        # ---------------- Stage C: output projection (partial) ----------------
        with tc.tile_pool(name="wop", bufs=1) as wopool, \
             tc.tile_pool(name="acp", bufs=2) as acpool, \
             tc.tile_pool(name="yop", bufs=3) as yopool, \
             tc.tile_pool(name="psC", bufs=4, space="PSUM") as psc:
            wo_sb = wopool.tile([128, hpc * d], BF16, name="wo_sb")
            for i in range(hpc):
                nc.sync.dma_start(out=wo_sb[:, i * d:(i + 1) * d],
                                  in_=woT[i * 128:(i + 1) * 128, :])
            for sq in range(ns):
                a_sb = acpool.tile([128, hpc * 128], BF16, name="a_sb")
                for i in range(hpc):
                    nc.sync.dma_start(
                        out=a_sb[:, i * 128:(i + 1) * 128],
                        in_=aT_d[i * 128:(i + 1) * 128, sq * 128:(sq + 1) * 128])
                for jn in range(nj):
                    yps = psc.tile([128, jw], F32, name="yps")
                    for i in range(hpc):
                        nc.tensor.matmul(yps, a_sb[:, i * 128:(i + 1) * 128],
                                         wo_sb[:, i * d + jn * jw: i * d + (jn + 1) * jw],
                                         start=(i == 0), stop=(i == hpc - 1))
                    yo = yopool.tile([128, jw], F32, name="yo")
                    nc.vector.tensor_copy(yo, yps)
                    nc.sync.dma_start(
                        out=y[sq * 128:(sq + 1) * 128, jn * jw:(jn + 1) * jw], in_=yo)
    return nc


# ---------------------------------------------------------------------------
# Host-side sharding + gather
# ---------------------------------------------------------------------------

_PERM_HEAD = np.concatenate([np.arange(0, HD, 2), np.arange(1, HD, 2)])


def _prep_in_maps(x, wq, wk, wv, wo, pos_cos, pos_sin, s=S, d=D, hpc=HPC):
    lf = hpc * HD
    h_total = d // HD
    groups = h_total // hpc
    # permute q/k feature rows within each head: even pairs first, then odd
    wq_p = wq.reshape(h_total, HD, d)[:, _PERM_HEAD, :].reshape(d, d)
    wk_p = wk.reshape(h_total, HD, d)[:, _PERM_HEAD, :].reshape(d, d)
    wqT_full = np.ascontiguousarray(wq_p.T).astype(NP_BF16)
    wkT_full = np.ascontiguousarray(wk_p.T).astype(NP_BF16)
    wvT_full = np.ascontiguousarray(wv.T).astype(NP_BF16)
    woT_full = np.ascontiguousarray(wo.T).astype(NP_BF16)
    cs_half = np.ascontiguousarray(pos_cos[0].T).astype(np.float32)  # [64, S]
    sn_half = np.ascontiguousarray(pos_sin[0].T).astype(np.float32)
    csd = np.concatenate([cs_half, cs_half], axis=0)
    snd = np.concatenate([sn_half, -sn_half], axis=0)
    in_maps = []
    n_batches = x.shape[0]
    for c in range(n_batches * groups):
        b, g = divmod(c, groups)
        in_maps.append({
            "xT": np.ascontiguousarray(x[b].T).astype(NP_BF16),
            "wqT": np.ascontiguousarray(wqT_full[:, g * lf:(g + 1) * lf]),
            "wkT": np.ascontiguousarray(wkT_full[:, g * lf:(g + 1) * lf]),
            "wvT": np.ascontiguousarray(wvT_full[:, g * lf:(g + 1) * lf]),
            "woT": np.ascontiguousarray(woT_full[g * lf:(g + 1) * lf, :]),
            "csd": csd,
            "snd": snd,
        })
    return in_maps


_NC_CACHE = {}


def _get_nc(s=S, d=D, hpc=HPC):
    key = (s, d, hpc)
    if key not in _NC_CACHE:
        _NC_CACHE[key] = build_nc(s, d, hpc)
    return _NC_CACHE[key]


def kernel(x, wq, wk, wv, wo, pos_cos, pos_sin):
    x = np.asarray(x, dtype=np.float32)
    in_maps = _prep_in_maps(np.asarray(x), np.asarray(wq), np.asarray(wk),
                            np.asarray(wv), np.asarray(wo),
                            np.asarray(pos_cos), np.asarray(pos_sin))
    nc = _get_nc()
    res = run_bass_kernel_spmd(nc, in_maps, core_ids=list(range(N_CORES)))
    out = np.empty((B, S, D), dtype=np.float32)
    for b in range(B):
        acc = res.results[b * GROUPS]["y"].astype(np.float32)
        for g in range(1, GROUPS):
            acc = acc + res.results[b * GROUPS + g]["y"]
        out[b] = acc
    return out
